# revision 17
# baseline (speedup 1.0000x reference)
"""Trainium2 Bass kernel for the GNN message-update MLP:

    out = relu(concat([v_i, v_j, e_ij], -1) @ W1 + b1) @ W2 + b2

Strategy (memory-bound, E = 1M edges, data-parallel across 8 cores):
  - Shard edges across the 8 NeuronCores (125000 each, padded to 126976).
  - Split-precision fp16 matmuls: every fp32 activation x ships as an
    (fp16 hi, fp16 lo) pair — same bytes as fp32 — and each logical fp32
    matmul x@w becomes xh@wh + xh@wl + xl@wh (the dropped lo@lo term is
    ~2^-22 relative). This runs the PE at full rate (1 cyc/row vs the 4x
    slower 2-pass fp32 mode) with ~5e-7 relative error vs the fp32 ref.
  - The three e_ij correction matmuls are K-stacked ([eh; eh; el], K=96)
    into ONE matmul per tile: 7 matmuls per 512-edge tile, all N=512.
  - Per 512-edge tile q (parity alternates PE column groups so output
    DMAs span all 128 partitions):
      layer1: 3x K=128 matmul + 1x K=96 e-matmul -> PSUM
      VectorE: hh = fp16(relu(psum + b1))   (tensor_scalar from PSUM)
      ScalarE: t  = relu(psum + b1)  fp32
      VectorE: hl = t - hh           fp16
      layer2: hh@w2h + hh@w2l + hl@w2h      -> PSUM
      ScalarE: out = psum (plain copy; b2 is added on host)
  - Host pre-packs transposed layouts so the device does only
    contiguous full-partition DMAs.
"""

import numpy as np

import concourse.bacc as bacc
import concourse.bass as bass
import concourse.mybir as mybir
import concourse.tile as tile
from concourse.bass_utils import run_bass_kernel_spmd

# ---- problem constants (hardcoded per harness contract) ----
E_TOTAL = 1_000_000
N_CORES = 8
IN_C = 64
IN_E = 32
HID = 64
OUT_C = 64

NHALF = 512                    # edges per matmul (moving free dim, 1 psum bank)
Q_PER_BLK = 8                  # 512-edge tiles per block
BLK_EDGES = NHALF * Q_PER_BLK  # 4096
EPC = E_TOTAL // N_CORES       # 125000 edges per core
N_BLK = -(-EPC // BLK_EDGES)   # 31
EPAD = N_BLK * BLK_EDGES       # 126976

F32 = mybir.dt.float32
F16 = mybir.dt.float16

# test.py hooks
_TRACE = False
LAST_RESULT = None

_PROGRAM_CACHE = {}


def _build_program():
    nc = bacc.Bacc(
        "TRN2",
        target_bir_lowering=False,
        debug=False,
        num_devices=N_CORES,
    )

    xta = nc.declare_dram_parameter(
        "xta", [N_BLK, 128, 2, BLK_EDGES], F16, isOutput=False
    )
    xtb = nc.declare_dram_parameter(
        "xtb", [N_BLK, 96, Q_PER_BLK, NHALF], F16, isOutput=False
    )
    w1a_h = nc.declare_dram_parameter("w1a_h", [128, 128], F16, isOutput=False)
    w1a_l = nc.declare_dram_parameter("w1a_l", [128, 128], F16, isOutput=False)
    w_es = nc.declare_dram_parameter("w_es", [96, 128], F16, isOutput=False)
    w2h_r = nc.declare_dram_parameter("w2h_r", [128, 128], F16, isOutput=False)
    w2l_r = nc.declare_dram_parameter("w2l_r", [128, 128], F16, isOutput=False)
    b1r = nc.declare_dram_parameter("b1r", [128, 1], F32, isOutput=False)
    out = nc.declare_dram_parameter(
        "out", [N_BLK, 128, 2, 2 * NHALF], F32, isOutput=True
    )

    with tile.TileContext(nc) as tc:
        with (
            tc.tile_pool(name="consts", bufs=1) as cpool,
            tc.tile_pool(name="xa", bufs=4) as xa_pool,
            tc.tile_pool(name="xb", bufs=4) as xb_pool,
            tc.tile_pool(name="hsp", bufs=4) as hsp_pool,
            tc.tile_pool(name="ob", bufs=3) as ob_pool,
            tc.tile_pool(name="ph", bufs=4, space="PSUM") as ph_pool,
            tc.tile_pool(name="po", bufs=4, space="PSUM") as po_pool,
        ):
            w1ah_t = cpool.tile([128, 128], F16)
            nc.sync.dma_start(w1ah_t[:], w1a_h[:])
            w1al_t = cpool.tile([128, 128], F16)
            nc.sync.dma_start(w1al_t[:], w1a_l[:])
            wes_t = cpool.tile([96, 128], F16)
            nc.sync.dma_start(wes_t[:], w_es[:])
            w2h_t = cpool.tile([128, 128], F16)
            nc.sync.dma_start(w2h_t[:], w2h_r[:])
            w2l_t = cpool.tile([128, 128], F16)
            nc.sync.dma_start(w2l_t[:], w2l_r[:])
            b1r_t = cpool.tile([128, 1], F32)
            nc.sync.dma_start(b1r_t[:], b1r[:])

            for blk in range(N_BLK):
                xa_t = xa_pool.tile([128, 2, BLK_EDGES], F16)
                half = BLK_EDGES // 2
                nc.sync.dma_start(
                    xa_t[:, :, 0:half], xta[blk][:, :, 0:half]
                )
                nc.sync.dma_start(
                    xa_t[:, :, half:BLK_EDGES], xta[blk][:, :, half:BLK_EDGES]
                )
                xb_t = xb_pool.tile([128, Q_PER_BLK, NHALF], F16, name="xb_t")
                nc.sync.dma_start(xb_t[0:96, :, :], xtb[blk])
                ob_t = ob_pool.tile([128, 2, 2 * NHALF], F32)

                for q in range(Q_PER_BLK):
                    grp, ql = divmod(q, 4)
                    par = ql % 2        # output partition half
                    cg = ql // 2        # output column half in ob[., grp]
                    c0 = 64 * par
                    if ql % 2 == 0:
                        hh_t = hsp_pool.tile([128, NHALF], F16, tag="hh", name="hh_t")
                        hl_t = hsp_pool.tile([128, NHALF], F16, tag="hl", name="hl_t")
                        t32_t = hsp_pool.tile([128, NHALF], F32, tag="t32", name="t32_t")
                    ph_t = ph_pool.tile([128, NHALF], F32)
                    xah = xa_t[:, 0, bass.ts(q, NHALF)]
                    xal = xa_t[:, 1, bass.ts(q, NHALF)]
                    # layer 1: xh@wh + xh@wl + xl@wh + e-stack
                    nc.tensor.matmul(
                        ph_t[:, :], w1ah_t[:, :], xah,
                        start=True, stop=False,
                    )
                    nc.tensor.matmul(
                        ph_t[:, :], w1al_t[:, :], xah,
                        start=False, stop=False,
                    )
                    nc.tensor.matmul(
                        ph_t[:, :], w1ah_t[:, :], xal,
                        start=False, stop=False,
                    )
                    nc.tensor.matmul(
                        ph_t[:, :], wes_t[:, :], xb_t[0:96, q, :],
                        start=False, stop=True,
                    )
                    # h = relu(psum + b1); fp16 hi directly on VectorE,
                    # exact fp32 on ScalarE, lo residual on VectorE
                    nc.vector.tensor_scalar(
                        hh_t[c0 : c0 + 64, :],
                        ph_t[c0 : c0 + 64, :],
                        b1r_t[c0 : c0 + 64, :],
                        0.0,
                        mybir.AluOpType.add,
                        mybir.AluOpType.max,
                    )
                    nc.scalar.activation(
                        t32_t[c0 : c0 + 64, :], ph_t[c0 : c0 + 64, :],
                        mybir.ActivationFunctionType.Relu,
                        bias=b1r_t[c0 : c0 + 64, :],
                    )
                    nc.vector.tensor_tensor(
                        hl_t[c0 : c0 + 64, :],
                        t32_t[c0 : c0 + 64, :],
                        hh_t[c0 : c0 + 64, :],
                        mybir.AluOpType.subtract,
                    )
                    # layer 2: hh@w2h + hh@w2l + hl@w2h
                    po_t = po_pool.tile([128, NHALF], F32)
                    nc.tensor.matmul(
                        po_t[:, :], w2h_t[c0 : c0 + 64, :],
                        hh_t[c0 : c0 + 64, :],
                        start=True, stop=False, tile_position=(c0, 0),
                    )
                    nc.tensor.matmul(
                        po_t[:, :], w2l_t[c0 : c0 + 64, :],
                        hh_t[c0 : c0 + 64, :],
                        start=False, stop=False, tile_position=(c0, 0),
                    )
                    nc.tensor.matmul(
                        po_t[:, :], w2h_t[c0 : c0 + 64, :],
                        hl_t[c0 : c0 + 64, :],
                        start=False, stop=True, tile_position=(c0, 0),
                    )
                    # PSUM -> SBUF copy on ScalarE (b2 is added on host)
                    nc.scalar.activation(
                        ob_t[c0 : c0 + 64, grp, bass.ts(cg, NHALF)],
                        po_t[c0 : c0 + 64, :],
                        mybir.ActivationFunctionType.Copy,
                    )
                nc.sync.dma_start(out[blk], ob_t[:])

    nc.compile()
    return nc


def _get_program():
    if "prog" not in _PROGRAM_CACHE:
        _PROGRAM_CACHE["prog"] = _build_program()
    return _PROGRAM_CACHE["prog"]


def _pad_rows(a, n):
    if a.shape[0] == n:
        return a
    pad = np.zeros((n - a.shape[0],) + a.shape[1:], dtype=a.dtype)
    return np.concatenate([a, pad], axis=0)


def _split16(a):
    """fp32 array -> (fp16 hi, fp16 lo) with hi + lo ~= a."""
    hi = a.astype(np.float16)
    lo = (a - hi.astype(np.float32)).astype(np.float16)
    return hi, lo


def _host_pack(v_i, v_j, e_ij, W1, b1, W2, b2):
    """Build per-core input maps in the device layouts."""
    v_i = np.ascontiguousarray(v_i, dtype=np.float32)
    v_j = np.ascontiguousarray(v_j, dtype=np.float32)
    e_ij = np.ascontiguousarray(e_ij, dtype=np.float32)

    Wx = np.asarray(W1[:128], dtype=np.float32)
    We = np.asarray(W1[128:160], dtype=np.float32)
    Wxh, Wxl = _split16(Wx)
    Weh, Wel = _split16(We)
    W2h, W2l = _split16(np.asarray(W2, dtype=np.float32))

    es_w = np.concatenate([Weh, Wel, Weh], axis=0)  # [96, 64] fp16

    def dup(w):  # [K, 64] -> [K, 128] column duplication (enables FWL)
        return np.ascontiguousarray(np.concatenate([w, w], axis=1))

    weights = {
        "w1a_h": dup(Wxh),
        "w1a_l": dup(Wxl),
        "w_es": dup(es_w),
        "w2h_r": dup(np.tile(W2h, (2, 1))),
        "w2l_r": dup(np.tile(W2l, (2, 1))),
        "b1r": np.ascontiguousarray(np.tile(b1, 2)[:, None], dtype=np.float32),
    }

    in_maps = []
    for c in range(N_CORES):
        sl = slice(c * EPC, (c + 1) * EPC)
        vi = _pad_rows(v_i[sl], EPAD)    # [EPAD, 64]
        vj = _pad_rows(v_j[sl], EPAD)
        ec = _pad_rows(e_ij[sl], EPAD)   # [EPAD, 32]

        # xta[b, p, h, n] = (Ah|Al)[p, b*4096 + n],  A = [v_i^T; v_j^T]
        A = np.concatenate([vi.T, vj.T], axis=0)          # [128, EPAD] f32
        Ah, Al = _split16(A)
        st = np.stack([Ah, Al], axis=1)                   # [128, 2, EPAD]
        xta = np.ascontiguousarray(
            st.reshape(128, 2, N_BLK, BLK_EDGES).transpose(2, 0, 1, 3)
        )  # [N_BLK, 128, 2, 4096] f16

        # e-stack [eh; eh; el] along K at rows 0:96 for every q
        eh, el = _split16(ec)                             # [EPAD, 32] each
        EST = np.concatenate([eh, eh, el], axis=1).T      # [96, EPAD] f16
        Tr = EST.reshape(96, N_BLK, Q_PER_BLK, NHALF)     # [r, b, q, n]
        xtb = np.ascontiguousarray(Tr.transpose(1, 0, 2, 3))

        in_maps.append({"xta": xta, "xtb": xtb, **weights})
    return in_maps


def _host_unpack(results, b2):
    """results: list of per-core dicts with 'out' [N_BLK, 128, 2, 1024]."""
    b2 = np.asarray(b2, dtype=np.float32)
    outs = []
    for c in range(N_CORES):
        o = np.asarray(results[c]["out"])
        # out[b, 64*par + p, grp, 512*cg + n]
        #   = OUT[b*4096 + grp*2048 + cg*1024 + par*512 + n, p]
        r = o.reshape(N_BLK, 2, 64, 2, 2, NHALF)   # [b, par, p, grp, cg, n]
        r = r.transpose(0, 3, 4, 1, 5, 2)           # [b, grp, cg, par, n, p]
        outs.append(np.ascontiguousarray(r).reshape(EPAD, OUT_C)[:EPC] + b2)
    return np.concatenate(outs, axis=0)


def kernel(v_i, v_j, e_ij, W1, b1, W2, b2):
    global LAST_RESULT
    nc = _get_program()
    in_maps = _host_pack(v_i, v_j, e_ij, W1, b1, W2, b2)
    res = run_bass_kernel_spmd(
        nc, in_maps, core_ids=list(range(N_CORES)), trace=_TRACE
    )
    LAST_RESULT = res
    return _host_unpack(res.results, b2)


# revision 18
# speedup vs baseline: 1.0207x; 1.0207x over previous
"""Trainium2 Bass kernel for the GNN message-update MLP:

    out = relu(concat([v_i, v_j, e_ij], -1) @ W1 + b1) @ W2 + b2

Strategy (memory-bound, E = 1M edges, data-parallel across 8 cores):
  - Shard edges across the 8 NeuronCores (125000 each, padded to 126976).
  - Split-precision fp16 matmuls: every fp32 activation x ships as an
    (fp16 hi, fp16 lo) pair — same bytes as fp32 — and each logical fp32
    matmul x@w becomes xh@wh + xh@wl + xl@wh (the dropped lo@lo term is
    ~2^-22 relative). This runs the PE at full rate (1 cyc/row vs the 4x
    slower 2-pass fp32 mode) with ~5e-7 relative error vs the fp32 ref.
  - The three e_ij correction matmuls are K-stacked ([eh; eh; el], K=96)
    into ONE matmul per tile: 7 matmuls per 512-edge tile, all N=512.
  - Per 512-edge tile q (parity alternates PE column groups so output
    DMAs span all 128 partitions):
      layer1: 3x K=128 matmul + 1x K=96 e-matmul -> PSUM
      VectorE: hh = fp16(relu(psum + b1))   (tensor_scalar from PSUM)
      ScalarE: t  = relu(psum + b1)  fp32
      VectorE: hl = t - hh           fp16
      layer2: hh@w2h + hh@w2l + hl@w2h      -> PSUM
      ScalarE: out = psum (plain copy; b2 is added on host)
  - Host pre-packs transposed layouts so the device does only
    contiguous full-partition DMAs.
"""

import numpy as np

import concourse.bacc as bacc
import concourse.bass as bass
import concourse.mybir as mybir
import concourse.tile as tile
from concourse.bass_utils import run_bass_kernel_spmd

# ---- problem constants (hardcoded per harness contract) ----
E_TOTAL = 1_000_000
N_CORES = 8
IN_C = 64
IN_E = 32
HID = 64
OUT_C = 64

NHALF = 512                    # edges per matmul (moving free dim, 1 psum bank)
Q_PER_BLK = 8                  # 512-edge tiles per block
BLK_EDGES = NHALF * Q_PER_BLK  # 4096
EPC = E_TOTAL // N_CORES       # 125000 edges per core
N_BLK = -(-EPC // BLK_EDGES)   # 31
EPAD = N_BLK * BLK_EDGES       # 126976

import os
import ml_dtypes

_HALF = os.environ.get("KERNEL_HALF", "bf16")
F32 = mybir.dt.float32
F16 = mybir.dt.bfloat16 if _HALF == "bf16" else mybir.dt.float16
_NP_HALF = ml_dtypes.bfloat16 if _HALF == "bf16" else np.float16

# test.py hooks
_TRACE = False
LAST_RESULT = None

_PROGRAM_CACHE = {}


def _build_program():
    nc = bacc.Bacc(
        "TRN2",
        target_bir_lowering=False,
        debug=False,
        num_devices=N_CORES,
    )

    xta = nc.declare_dram_parameter(
        "xta", [N_BLK, 128, 2, BLK_EDGES], F16, isOutput=False
    )
    xtb = nc.declare_dram_parameter(
        "xtb", [N_BLK, 96, Q_PER_BLK, NHALF], F16, isOutput=False
    )
    w1a_h = nc.declare_dram_parameter("w1a_h", [128, HID], F16, isOutput=False)
    w1a_l = nc.declare_dram_parameter("w1a_l", [128, HID], F16, isOutput=False)
    w_es = nc.declare_dram_parameter("w_es", [96, HID], F16, isOutput=False)
    w2h_r = nc.declare_dram_parameter("w2h_r", [128, OUT_C], F16, isOutput=False)
    w2l_r = nc.declare_dram_parameter("w2l_r", [128, OUT_C], F16, isOutput=False)
    b1r = nc.declare_dram_parameter("b1r", [128, 1], F32, isOutput=False)
    out = nc.declare_dram_parameter(
        "out", [N_BLK, 128, 2, 2 * NHALF], F32, isOutput=True
    )

    with tile.TileContext(nc) as tc:
        with (
            tc.tile_pool(name="consts", bufs=1) as cpool,
            tc.tile_pool(name="xa", bufs=3) as xa_pool,
            tc.tile_pool(name="xb", bufs=3) as xb_pool,
            tc.tile_pool(name="hsp", bufs=4) as hsp_pool,
            tc.tile_pool(name="ob", bufs=3) as ob_pool,
            tc.tile_pool(name="ph", bufs=4, space="PSUM") as ph_pool,
            tc.tile_pool(name="po", bufs=4, space="PSUM") as po_pool,
        ):
            w1ah_t = cpool.tile([128, HID], F16)
            nc.sync.dma_start(w1ah_t[:], w1a_h[:])
            w1al_t = cpool.tile([128, HID], F16)
            nc.sync.dma_start(w1al_t[:], w1a_l[:])
            wes_t = cpool.tile([96, HID], F16)
            nc.sync.dma_start(wes_t[:], w_es[:])
            w2h_t = cpool.tile([128, OUT_C], F16)
            nc.sync.dma_start(w2h_t[:], w2h_r[:])
            w2l_t = cpool.tile([128, OUT_C], F16)
            nc.sync.dma_start(w2l_t[:], w2l_r[:])
            b1r_t = cpool.tile([128, 1], F32)
            nc.sync.dma_start(b1r_t[:], b1r[:])

            for blk in range(N_BLK):
                xa_t = xa_pool.tile([128, 2, BLK_EDGES], F16)
                nc.sync.dma_start(xa_t[:], xta[blk])
                xb_t = xb_pool.tile([128, Q_PER_BLK, NHALF], F16, name="xb_t")
                nc.sync.dma_start(xb_t[0:96, :, :], xtb[blk])
                ob_t = ob_pool.tile([128, 2, 2 * NHALF], F32)

                for q in range(Q_PER_BLK):
                    grp, ql = divmod(q, 4)
                    par = ql % 2        # output partition half
                    cg = ql // 2        # output column half in ob[., grp]
                    c0 = 64 * par
                    if ql % 2 == 0:
                        hh_t = hsp_pool.tile([128, NHALF], F16, tag="hh", name="hh_t")
                        hl_t = hsp_pool.tile([128, NHALF], F16, tag="hl", name="hl_t")
                        t32_t = hsp_pool.tile([128, NHALF], F32, tag="t32", name="t32_t")
                    ph_t = ph_pool.tile([128, NHALF], F32)
                    xah = xa_t[:, 0, bass.ts(q, NHALF)]
                    xal = xa_t[:, 1, bass.ts(q, NHALF)]
                    # layer 1: xh@wh + xh@wl + xl@wh + e-stack
                    nc.tensor.matmul(
                        ph_t[c0 : c0 + 64, :], w1ah_t[:, :], xah,
                        start=True, stop=False, tile_position=(0, c0),
                    )
                    nc.tensor.matmul(
                        ph_t[c0 : c0 + 64, :], w1al_t[:, :], xah,
                        start=False, stop=False, tile_position=(0, c0),
                    )
                    nc.tensor.matmul(
                        ph_t[c0 : c0 + 64, :], w1ah_t[:, :], xal,
                        start=False, stop=False, tile_position=(0, c0),
                    )
                    nc.tensor.matmul(
                        ph_t[c0 : c0 + 64, :], wes_t[:, :], xb_t[0:96, q, :],
                        start=False, stop=True, tile_position=(0, c0),
                    )
                    # h = relu(psum + b1); fp16 hi directly on VectorE,
                    # exact fp32 on ScalarE, lo residual on VectorE
                    nc.vector.tensor_scalar(
                        hh_t[c0 : c0 + 64, :],
                        ph_t[c0 : c0 + 64, :],
                        b1r_t[c0 : c0 + 64, :],
                        0.0,
                        mybir.AluOpType.add,
                        mybir.AluOpType.max,
                    )
                    nc.scalar.activation(
                        t32_t[c0 : c0 + 64, :], ph_t[c0 : c0 + 64, :],
                        mybir.ActivationFunctionType.Relu,
                        bias=b1r_t[c0 : c0 + 64, :],
                    )
                    nc.vector.tensor_tensor(
                        hl_t[c0 : c0 + 64, :],
                        t32_t[c0 : c0 + 64, :],
                        hh_t[c0 : c0 + 64, :],
                        mybir.AluOpType.subtract,
                    )
                    # layer 2: hh@w2h + hh@w2l + hl@w2h
                    po_t = po_pool.tile([128, NHALF], F32)
                    nc.tensor.matmul(
                        po_t[c0 : c0 + 64, :], w2h_t[c0 : c0 + 64, :],
                        hh_t[c0 : c0 + 64, :],
                        start=True, stop=False, tile_position=(c0, c0),
                    )
                    nc.tensor.matmul(
                        po_t[c0 : c0 + 64, :], w2l_t[c0 : c0 + 64, :],
                        hh_t[c0 : c0 + 64, :],
                        start=False, stop=False, tile_position=(c0, c0),
                    )
                    nc.tensor.matmul(
                        po_t[c0 : c0 + 64, :], w2h_t[c0 : c0 + 64, :],
                        hl_t[c0 : c0 + 64, :],
                        start=False, stop=True, tile_position=(c0, c0),
                    )
                    # PSUM -> SBUF copy on ScalarE (b2 is added on host)
                    nc.scalar.activation(
                        ob_t[c0 : c0 + 64, grp, bass.ts(cg, NHALF)],
                        po_t[c0 : c0 + 64, :],
                        mybir.ActivationFunctionType.Copy,
                    )
                nc.sync.dma_start(out[blk], ob_t[:])

    nc.compile()
    return nc


def _get_program():
    if "prog" not in _PROGRAM_CACHE:
        _PROGRAM_CACHE["prog"] = _build_program()
    return _PROGRAM_CACHE["prog"]


def _pad_rows(a, n):
    if a.shape[0] == n:
        return a
    pad = np.zeros((n - a.shape[0],) + a.shape[1:], dtype=a.dtype)
    return np.concatenate([a, pad], axis=0)


def _split16(a):
    """fp32 array -> (half hi, half lo) with hi + lo ~= a."""
    hi = a.astype(_NP_HALF)
    lo = (a - hi.astype(np.float32)).astype(_NP_HALF)
    return hi, lo


def _host_pack(v_i, v_j, e_ij, W1, b1, W2, b2):
    """Build per-core input maps in the device layouts."""
    v_i = np.ascontiguousarray(v_i, dtype=np.float32)
    v_j = np.ascontiguousarray(v_j, dtype=np.float32)
    e_ij = np.ascontiguousarray(e_ij, dtype=np.float32)

    Wx = np.asarray(W1[:128], dtype=np.float32)
    We = np.asarray(W1[128:160], dtype=np.float32)
    Wxh, Wxl = _split16(Wx)
    Weh, Wel = _split16(We)
    W2h, W2l = _split16(np.asarray(W2, dtype=np.float32))

    es_w = np.concatenate([Weh, Wel, Weh], axis=0)  # [96, 64] halfword

    weights = {
        "w1a_h": np.ascontiguousarray(Wxh),
        "w1a_l": np.ascontiguousarray(Wxl),
        "w_es": np.ascontiguousarray(es_w),
        "w2h_r": np.ascontiguousarray(np.tile(W2h, (2, 1))),
        "w2l_r": np.ascontiguousarray(np.tile(W2l, (2, 1))),
        "b1r": np.ascontiguousarray(np.tile(b1, 2)[:, None], dtype=np.float32),
    }

    in_maps = []
    for c in range(N_CORES):
        sl = slice(c * EPC, (c + 1) * EPC)
        vi = _pad_rows(v_i[sl], EPAD)    # [EPAD, 64]
        vj = _pad_rows(v_j[sl], EPAD)
        ec = _pad_rows(e_ij[sl], EPAD)   # [EPAD, 32]

        # xta[b, p, h, n] = (Ah|Al)[p, b*4096 + n],  A = [v_i^T; v_j^T]
        A = np.concatenate([vi.T, vj.T], axis=0)          # [128, EPAD] f32
        Ah, Al = _split16(A)
        st = np.stack([Ah, Al], axis=1)                   # [128, 2, EPAD]
        xta = np.ascontiguousarray(
            st.reshape(128, 2, N_BLK, BLK_EDGES).transpose(2, 0, 1, 3)
        )  # [N_BLK, 128, 2, 4096] half

        # e-stack [eh; eh; el] along K at rows 0:96 for every q
        eh, el = _split16(ec)                             # [EPAD, 32] each
        EST = np.concatenate([eh, eh, el], axis=1).T      # [96, EPAD] f16
        Tr = EST.reshape(96, N_BLK, Q_PER_BLK, NHALF)     # [r, b, q, n]
        xtb = np.ascontiguousarray(Tr.transpose(1, 0, 2, 3))

        in_maps.append({"xta": xta, "xtb": xtb, **weights})
    return in_maps


def _host_unpack(results, b2):
    """results: list of per-core dicts with 'out' [N_BLK, 128, 2, 1024]."""
    b2 = np.asarray(b2, dtype=np.float32)
    outs = []
    for c in range(N_CORES):
        o = np.asarray(results[c]["out"])
        # out[b, 64*par + p, grp, 512*cg + n]
        #   = OUT[b*4096 + grp*2048 + cg*1024 + par*512 + n, p]
        r = o.reshape(N_BLK, 2, 64, 2, 2, NHALF)   # [b, par, p, grp, cg, n]
        r = r.transpose(0, 3, 4, 1, 5, 2)           # [b, grp, cg, par, n, p]
        outs.append(np.ascontiguousarray(r).reshape(EPAD, OUT_C)[:EPC] + b2)
    return np.concatenate(outs, axis=0)


def kernel(v_i, v_j, e_ij, W1, b1, W2, b2):
    global LAST_RESULT
    nc = _get_program()
    in_maps = _host_pack(v_i, v_j, e_ij, W1, b1, W2, b2)
    res = run_bass_kernel_spmd(
        nc, in_maps, core_ids=list(range(N_CORES)), trace=_TRACE
    )
    LAST_RESULT = res
    return _host_unpack(res.results, b2)


# revision 20
# speedup vs baseline: 1.0834x; 1.0614x over previous
"""Trainium2 Bass kernel for the GNN message-update MLP:

    out = relu(concat([v_i, v_j, e_ij], -1) @ W1 + b1) @ W2 + b2

Strategy (memory-bound, E = 1M edges, data-parallel across 8 cores):
  - Shard edges across the 8 NeuronCores (125000 each, padded to 126976).
  - Split-precision fp16 matmuls: every fp32 activation x ships as an
    (fp16 hi, fp16 lo) pair — same bytes as fp32 — and each logical fp32
    matmul x@w becomes xh@wh + xh@wl + xl@wh (the dropped lo@lo term is
    ~2^-22 relative). This runs the PE at full rate (1 cyc/row vs the 4x
    slower 2-pass fp32 mode) with ~5e-7 relative error vs the fp32 ref.
  - The three e_ij correction matmuls are K-stacked ([eh; eh; el], K=96)
    into ONE matmul per tile: 7 matmuls per 512-edge tile, all N=512.
  - Per 512-edge tile q (parity alternates PE column groups so output
    DMAs span all 128 partitions):
      layer1: 3x K=128 matmul + 1x K=96 e-matmul -> PSUM
      VectorE: hh = fp16(relu(psum + b1))   (tensor_scalar from PSUM)
      ScalarE: t  = relu(psum + b1)  fp32
      VectorE: hl = t - hh           fp16
      layer2: hh@w2h + hh@w2l + hl@w2h      -> PSUM
      ScalarE: out = psum (plain copy; b2 is added on host)
  - Host pre-packs transposed layouts so the device does only
    contiguous full-partition DMAs.
"""

import numpy as np

import concourse.bacc as bacc
import concourse.bass as bass
import concourse.mybir as mybir
import concourse.tile as tile
from concourse.bass_utils import run_bass_kernel_spmd

# ---- problem constants (hardcoded per harness contract) ----
E_TOTAL = 1_000_000
N_CORES = 8
IN_C = 64
IN_E = 32
HID = 64
OUT_C = 64

NHALF = 512                    # edges per matmul (moving free dim, 1 psum bank)
Q_PER_BLK = 8                  # 512-edge tiles per block
BLK_EDGES = NHALF * Q_PER_BLK  # 4096
EPC = E_TOTAL // N_CORES       # 125000 edges per core
N_BLK = -(-EPC // BLK_EDGES)   # 31
EPAD = N_BLK * BLK_EDGES       # 126976

import os
import ml_dtypes

_HALF = os.environ.get("KERNEL_HALF", "fp16")
F32 = mybir.dt.float32
F16 = mybir.dt.bfloat16 if _HALF == "bf16" else mybir.dt.float16
_NP_HALF = ml_dtypes.bfloat16 if _HALF == "bf16" else np.float16

# test.py hooks
_TRACE = False
LAST_RESULT = None

_PROGRAM_CACHE = {}


def _build_program():
    nc = bacc.Bacc(
        "TRN2",
        target_bir_lowering=False,
        debug=False,
        num_devices=N_CORES,
    )

    xta = nc.declare_dram_parameter(
        "xta", [N_BLK, 128, 2, BLK_EDGES], F16, isOutput=False
    )
    xtb = nc.declare_dram_parameter(
        "xtb", [N_BLK, 96, Q_PER_BLK, NHALF], F16, isOutput=False
    )
    w1a_h = nc.declare_dram_parameter("w1a_h", [128, HID], F16, isOutput=False)
    w1a_l = nc.declare_dram_parameter("w1a_l", [128, HID], F16, isOutput=False)
    w_es = nc.declare_dram_parameter("w_es", [96, HID], F16, isOutput=False)
    w2h_r = nc.declare_dram_parameter("w2h_r", [128, OUT_C], F16, isOutput=False)
    w2l_r = nc.declare_dram_parameter("w2l_r", [128, OUT_C], F16, isOutput=False)
    b1r = nc.declare_dram_parameter("b1r", [128, 1], F32, isOutput=False)
    out = nc.declare_dram_parameter(
        "out", [N_BLK, 128, 2, 2 * NHALF], F32, isOutput=True
    )

    with tile.TileContext(nc) as tc:
        with (
            tc.tile_pool(name="consts", bufs=1) as cpool,
            tc.tile_pool(name="xa", bufs=3) as xa_pool,
            tc.tile_pool(name="xb", bufs=3) as xb_pool,
            tc.tile_pool(name="hsp", bufs=4) as hsp_pool,
            tc.tile_pool(name="ob", bufs=3) as ob_pool,
            tc.tile_pool(name="ph", bufs=4, space="PSUM") as ph_pool,
            tc.tile_pool(name="po", bufs=4, space="PSUM") as po_pool,
        ):
            w1ah_t = cpool.tile([128, HID], F16)
            nc.sync.dma_start(w1ah_t[:], w1a_h[:])
            w1al_t = cpool.tile([128, HID], F16)
            nc.sync.dma_start(w1al_t[:], w1a_l[:])
            wes_t = cpool.tile([96, HID], F16)
            nc.sync.dma_start(wes_t[:], w_es[:])
            w2h_t = cpool.tile([128, OUT_C], F16)
            nc.sync.dma_start(w2h_t[:], w2h_r[:])
            w2l_t = cpool.tile([128, OUT_C], F16)
            nc.sync.dma_start(w2l_t[:], w2l_r[:])
            b1r_t = cpool.tile([128, 1], F32)
            nc.sync.dma_start(b1r_t[:], b1r[:])

            warm_t = cpool.tile([128, NHALF], F16)
            nc.vector.memset(warm_t[:], 0.0)
            warm_ps = ph_pool.tile([128, NHALF], F32, tag="ph_t", name="warm_ps")
            for _ in range(56):
                nc.tensor.matmul(
                    warm_ps[:, :], warm_t[:, 0:128], warm_t[:, :],
                    start=True, stop=True,
                )

            for blk in range(N_BLK):
                xa_t = xa_pool.tile([128, 2, BLK_EDGES], F16)
                nc.sync.dma_start(xa_t[:], xta[blk])
                xb_t = xb_pool.tile([128, Q_PER_BLK, NHALF], F16, name="xb_t")
                nc.sync.dma_start(xb_t[0:96, :, :], xtb[blk])
                ob_t = ob_pool.tile([128, 2, 2 * NHALF], F32)

                for q in range(Q_PER_BLK):
                    grp, ql = divmod(q, 4)
                    par = ql % 2        # output partition half
                    cg = ql // 2        # output column half in ob[., grp]
                    c0 = 64 * par
                    if ql % 2 == 0:
                        hh_t = hsp_pool.tile([128, NHALF], F16, tag="hh", name="hh_t")
                        hl_t = hsp_pool.tile([128, NHALF], F16, tag="hl", name="hl_t")
                        t32_t = hsp_pool.tile([128, NHALF], F32, tag="t32", name="t32_t")
                    ph_t = ph_pool.tile([128, NHALF], F32)
                    xah = xa_t[:, 0, bass.ts(q, NHALF)]
                    xal = xa_t[:, 1, bass.ts(q, NHALF)]
                    # layer 1: xh@wh + xh@wl + xl@wh + e-stack
                    nc.tensor.matmul(
                        ph_t[c0 : c0 + 64, :], w1ah_t[:, :], xah,
                        start=True, stop=False, tile_position=(0, c0),
                    )
                    nc.tensor.matmul(
                        ph_t[c0 : c0 + 64, :], w1al_t[:, :], xah,
                        start=False, stop=False, tile_position=(0, c0),
                    )
                    nc.tensor.matmul(
                        ph_t[c0 : c0 + 64, :], w1ah_t[:, :], xal,
                        start=False, stop=False, tile_position=(0, c0),
                    )
                    nc.tensor.matmul(
                        ph_t[c0 : c0 + 64, :], wes_t[:, :], xb_t[0:96, q, :],
                        start=False, stop=True, tile_position=(0, c0),
                    )
                    # h = relu(psum + b1); fp16 hi directly on VectorE,
                    # exact fp32 on ScalarE, lo residual on VectorE
                    nc.vector.tensor_scalar(
                        hh_t[c0 : c0 + 64, :],
                        ph_t[c0 : c0 + 64, :],
                        b1r_t[c0 : c0 + 64, :],
                        0.0,
                        mybir.AluOpType.add,
                        mybir.AluOpType.max,
                    )
                    nc.scalar.activation(
                        t32_t[c0 : c0 + 64, :], ph_t[c0 : c0 + 64, :],
                        mybir.ActivationFunctionType.Relu,
                        bias=b1r_t[c0 : c0 + 64, :],
                    )
                    nc.vector.tensor_tensor(
                        hl_t[c0 : c0 + 64, :],
                        t32_t[c0 : c0 + 64, :],
                        hh_t[c0 : c0 + 64, :],
                        mybir.AluOpType.subtract,
                    )
                    # layer 2: hh@w2h + hh@w2l + hl@w2h
                    po_t = po_pool.tile([128, NHALF], F32)
                    nc.tensor.matmul(
                        po_t[c0 : c0 + 64, :], w2h_t[c0 : c0 + 64, :],
                        hh_t[c0 : c0 + 64, :],
                        start=True, stop=False, tile_position=(c0, c0),
                    )
                    nc.tensor.matmul(
                        po_t[c0 : c0 + 64, :], w2l_t[c0 : c0 + 64, :],
                        hh_t[c0 : c0 + 64, :],
                        start=False, stop=False, tile_position=(c0, c0),
                    )
                    nc.tensor.matmul(
                        po_t[c0 : c0 + 64, :], w2h_t[c0 : c0 + 64, :],
                        hl_t[c0 : c0 + 64, :],
                        start=False, stop=True, tile_position=(c0, c0),
                    )
                    # PSUM -> SBUF copy on ScalarE (b2 is added on host)
                    nc.scalar.activation(
                        ob_t[c0 : c0 + 64, grp, bass.ts(cg, NHALF)],
                        po_t[c0 : c0 + 64, :],
                        mybir.ActivationFunctionType.Copy,
                    )
                nc.sync.dma_start(out[blk], ob_t[:])

    nc.compile()
    return nc


def _get_program():
    if "prog" not in _PROGRAM_CACHE:
        _PROGRAM_CACHE["prog"] = _build_program()
    return _PROGRAM_CACHE["prog"]


def _pad_rows(a, n):
    if a.shape[0] == n:
        return a
    pad = np.zeros((n - a.shape[0],) + a.shape[1:], dtype=a.dtype)
    return np.concatenate([a, pad], axis=0)


def _split16(a):
    """fp32 array -> (half hi, half lo) with hi + lo ~= a."""
    hi = a.astype(_NP_HALF)
    lo = (a - hi.astype(np.float32)).astype(_NP_HALF)
    return hi, lo


def _host_pack(v_i, v_j, e_ij, W1, b1, W2, b2):
    """Build per-core input maps in the device layouts."""
    v_i = np.ascontiguousarray(v_i, dtype=np.float32)
    v_j = np.ascontiguousarray(v_j, dtype=np.float32)
    e_ij = np.ascontiguousarray(e_ij, dtype=np.float32)

    Wx = np.asarray(W1[:128], dtype=np.float32)
    We = np.asarray(W1[128:160], dtype=np.float32)
    Wxh, Wxl = _split16(Wx)
    Weh, Wel = _split16(We)
    W2h, W2l = _split16(np.asarray(W2, dtype=np.float32))

    es_w = np.concatenate([Weh, Wel, Weh], axis=0)  # [96, 64] halfword

    weights = {
        "w1a_h": np.ascontiguousarray(Wxh),
        "w1a_l": np.ascontiguousarray(Wxl),
        "w_es": np.ascontiguousarray(es_w),
        "w2h_r": np.ascontiguousarray(np.tile(W2h, (2, 1))),
        "w2l_r": np.ascontiguousarray(np.tile(W2l, (2, 1))),
        "b1r": np.ascontiguousarray(np.tile(b1, 2)[:, None], dtype=np.float32),
    }

    in_maps = []
    for c in range(N_CORES):
        sl = slice(c * EPC, (c + 1) * EPC)
        vi = _pad_rows(v_i[sl], EPAD)    # [EPAD, 64]
        vj = _pad_rows(v_j[sl], EPAD)
        ec = _pad_rows(e_ij[sl], EPAD)   # [EPAD, 32]

        # xta[b, p, h, n] = (Ah|Al)[p, b*4096 + n],  A = [v_i^T; v_j^T]
        A = np.concatenate([vi.T, vj.T], axis=0)          # [128, EPAD] f32
        Ah, Al = _split16(A)
        st = np.stack([Ah, Al], axis=1)                   # [128, 2, EPAD]
        xta = np.ascontiguousarray(
            st.reshape(128, 2, N_BLK, BLK_EDGES).transpose(2, 0, 1, 3)
        )  # [N_BLK, 128, 2, 4096] half

        # e-stack [eh; eh; el] along K at rows 0:96 for every q
        eh, el = _split16(ec)                             # [EPAD, 32] each
        EST = np.concatenate([eh, eh, el], axis=1).T      # [96, EPAD] f16
        Tr = EST.reshape(96, N_BLK, Q_PER_BLK, NHALF)     # [r, b, q, n]
        xtb = np.ascontiguousarray(Tr.transpose(1, 0, 2, 3))

        in_maps.append({"xta": xta, "xtb": xtb, **weights})
    return in_maps


def _host_unpack(results, b2):
    """results: list of per-core dicts with 'out' [N_BLK, 128, 2, 1024]."""
    b2 = np.asarray(b2, dtype=np.float32)
    outs = []
    for c in range(N_CORES):
        o = np.asarray(results[c]["out"])
        # out[b, 64*par + p, grp, 512*cg + n]
        #   = OUT[b*4096 + grp*2048 + cg*1024 + par*512 + n, p]
        r = o.reshape(N_BLK, 2, 64, 2, 2, NHALF)   # [b, par, p, grp, cg, n]
        r = r.transpose(0, 3, 4, 1, 5, 2)           # [b, grp, cg, par, n, p]
        outs.append(np.ascontiguousarray(r).reshape(EPAD, OUT_C)[:EPC] + b2)
    return np.concatenate(outs, axis=0)


def kernel(v_i, v_j, e_ij, W1, b1, W2, b2):
    global LAST_RESULT
    nc = _get_program()
    in_maps = _host_pack(v_i, v_j, e_ij, W1, b1, W2, b2)
    res = run_bass_kernel_spmd(
        nc, in_maps, core_ids=list(range(N_CORES)), trace=_TRACE
    )
    LAST_RESULT = res
    return _host_unpack(res.results, b2)


# revision 21
# speedup vs baseline: 1.0860x; 1.0024x over previous
"""Trainium2 Bass kernel for the GNN message-update MLP:

    out = relu(concat([v_i, v_j, e_ij], -1) @ W1 + b1) @ W2 + b2

Strategy (memory-bound, E = 1M edges, data-parallel across 8 cores):
  - Shard edges across the 8 NeuronCores (125000 each, padded to 126976).
  - Split-precision fp16 matmuls: every fp32 activation x ships as an
    (fp16 hi, fp16 lo) pair — same bytes as fp32 — and each logical fp32
    matmul x@w becomes xh@wh + xh@wl + xl@wh (the dropped lo@lo term is
    ~2^-22 relative). This runs the PE at full rate (1 cyc/row vs the 4x
    slower 2-pass fp32 mode) with ~5e-7 relative error vs the fp32 ref.
  - The three e_ij correction matmuls are K-stacked ([eh; eh; el], K=96)
    into ONE matmul per tile: 7 matmuls per 512-edge tile, all N=512.
  - Per 512-edge tile q (parity alternates PE column groups so output
    DMAs span all 128 partitions):
      layer1: 3x K=128 matmul + 1x K=96 e-matmul -> PSUM
      VectorE: hh = fp16(relu(psum + b1))   (tensor_scalar from PSUM)
      ScalarE: t  = relu(psum + b1)  fp32
      VectorE: hl = t - hh           fp16
      layer2: hh@w2h + hh@w2l + hl@w2h      -> PSUM
      ScalarE: out = psum (plain copy; b2 is added on host)
  - Host pre-packs transposed layouts so the device does only
    contiguous full-partition DMAs.
"""

import numpy as np

import concourse.bacc as bacc
import concourse.bass as bass
import concourse.mybir as mybir
import concourse.tile as tile
from concourse.bass_utils import run_bass_kernel_spmd

# ---- problem constants (hardcoded per harness contract) ----
E_TOTAL = 1_000_000
N_CORES = 8
IN_C = 64
IN_E = 32
HID = 64
OUT_C = 64

NHALF = 512                    # edges per matmul (moving free dim, 1 psum bank)
Q_PER_BLK = 8                  # 512-edge tiles per block
BLK_EDGES = NHALF * Q_PER_BLK  # 4096
EPC = E_TOTAL // N_CORES       # 125000 edges per core
N_BLK = -(-EPC // BLK_EDGES)   # 31
EPAD = N_BLK * BLK_EDGES       # 126976

import os
import ml_dtypes

_HALF = os.environ.get("KERNEL_HALF", "fp16")
F32 = mybir.dt.float32
F16 = mybir.dt.bfloat16 if _HALF == "bf16" else mybir.dt.float16
_NP_HALF = ml_dtypes.bfloat16 if _HALF == "bf16" else np.float16

# test.py hooks
_TRACE = False
LAST_RESULT = None

_PROGRAM_CACHE = {}


def _build_program():
    nc = bacc.Bacc(
        "TRN2",
        target_bir_lowering=False,
        debug=False,
        num_devices=N_CORES,
    )

    xta = nc.declare_dram_parameter(
        "xta", [N_BLK, 128, 2, BLK_EDGES], F16, isOutput=False
    )
    xtb = nc.declare_dram_parameter(
        "xtb", [N_BLK, 96, Q_PER_BLK, NHALF], F16, isOutput=False
    )
    w1a_h = nc.declare_dram_parameter("w1a_h", [128, HID], F16, isOutput=False)
    w1a_l = nc.declare_dram_parameter("w1a_l", [128, HID], F16, isOutput=False)
    w_es = nc.declare_dram_parameter("w_es", [96, HID], F16, isOutput=False)
    w2h_r = nc.declare_dram_parameter("w2h_r", [128, OUT_C], F16, isOutput=False)
    w2l_r = nc.declare_dram_parameter("w2l_r", [128, OUT_C], F16, isOutput=False)
    b1r = nc.declare_dram_parameter("b1r", [128, 1], F32, isOutput=False)
    out = nc.declare_dram_parameter(
        "out", [N_BLK, 128, 2, 2 * NHALF], F32, isOutput=True
    )

    with tile.TileContext(nc) as tc:
        with (
            tc.tile_pool(name="consts", bufs=1) as cpool,
            tc.tile_pool(name="xa", bufs=3) as xa_pool,
            tc.tile_pool(name="xb", bufs=3) as xb_pool,
            tc.tile_pool(name="hsp", bufs=4) as hsp_pool,
            tc.tile_pool(name="ob", bufs=3) as ob_pool,
            tc.tile_pool(name="ph", bufs=4, space="PSUM") as ph_pool,
            tc.tile_pool(name="po", bufs=4, space="PSUM") as po_pool,
        ):
            w1ah_t = cpool.tile([128, HID], F16)
            nc.sync.dma_start(w1ah_t[:], w1a_h[:])
            w1al_t = cpool.tile([128, HID], F16)
            nc.sync.dma_start(w1al_t[:], w1a_l[:])
            wes_t = cpool.tile([96, HID], F16)
            nc.sync.dma_start(wes_t[:], w_es[:])
            w2h_t = cpool.tile([128, OUT_C], F16)
            nc.sync.dma_start(w2h_t[:], w2h_r[:])
            w2l_t = cpool.tile([128, OUT_C], F16)
            nc.sync.dma_start(w2l_t[:], w2l_r[:])
            b1r_t = cpool.tile([128, 1], F32)
            nc.sync.dma_start(b1r_t[:], b1r[:])

            warm_t = cpool.tile([128, NHALF], F16)
            nc.vector.memset(warm_t[:], 0.0)
            warm_ps = ph_pool.tile([128, NHALF], F32, tag="ph_t", name="warm_ps")
            for _ in range(56):
                nc.tensor.matmul(
                    warm_ps[:, :], warm_t[:, 0:128], warm_t[:, :],
                    start=True, stop=True,
                )

            for blk in range(N_BLK):
                xa_t = xa_pool.tile([128, 2, BLK_EDGES], F16)
                nc.sync.dma_start(xa_t[:], xta[blk])
                xb_t = xb_pool.tile([128, Q_PER_BLK, NHALF], F16, name="xb_t")
                nc.sync.dma_start(xb_t[0:96, :, :], xtb[blk])
                ob_t = ob_pool.tile([128, 2, 2 * NHALF], F32)

                for pair in range(Q_PER_BLK // 2):
                    hh_t = hsp_pool.tile([128, NHALF], F16, tag="hh", name="hh_t")
                    hl_t = hsp_pool.tile([128, NHALF], F16, tag="hl", name="hl_t")
                    t32_t = hsp_pool.tile([128, NHALF], F32, tag="t32", name="t32_t")
                    ph = [None, None]
                    po = [None, None]
                    # layer 1 for both tiles of the pair
                    for par in range(2):
                        q = 2 * pair + par
                        c0 = 64 * par
                        ph_t = ph_pool.tile([128, NHALF], F32, name="ph_t")
                        ph[par] = ph_t
                        xah = xa_t[:, 0, bass.ts(q, NHALF)]
                        xal = xa_t[:, 1, bass.ts(q, NHALF)]
                        nc.tensor.matmul(
                            ph_t[c0 : c0 + 64, :], w1ah_t[:, :], xah,
                            start=True, stop=False, tile_position=(0, c0),
                        )
                        nc.tensor.matmul(
                            ph_t[c0 : c0 + 64, :], w1al_t[:, :], xah,
                            start=False, stop=False, tile_position=(0, c0),
                        )
                        nc.tensor.matmul(
                            ph_t[c0 : c0 + 64, :], w1ah_t[:, :], xal,
                            start=False, stop=False, tile_position=(0, c0),
                        )
                        nc.tensor.matmul(
                            ph_t[c0 : c0 + 64, :], wes_t[:, :], xb_t[0:96, q, :],
                            start=False, stop=True, tile_position=(0, c0),
                        )
                    # h split ops for both tiles
                    for par in range(2):
                        c0 = 64 * par
                        ph_t = ph[par]
                        nc.vector.tensor_scalar(
                            hh_t[c0 : c0 + 64, :],
                            ph_t[c0 : c0 + 64, :],
                            b1r_t[c0 : c0 + 64, :],
                            0.0,
                            mybir.AluOpType.add,
                            mybir.AluOpType.max,
                        )
                        nc.scalar.activation(
                            t32_t[c0 : c0 + 64, :], ph_t[c0 : c0 + 64, :],
                            mybir.ActivationFunctionType.Relu,
                            bias=b1r_t[c0 : c0 + 64, :],
                        )
                        nc.vector.tensor_tensor(
                            hl_t[c0 : c0 + 64, :],
                            t32_t[c0 : c0 + 64, :],
                            hh_t[c0 : c0 + 64, :],
                            mybir.AluOpType.subtract,
                        )
                        po[par] = po_pool.tile([128, NHALF], F32, name="po_t")
                    # layer 2 interleaved: the two tiles use disjoint PE
                    # row AND column groups, so adjacent matmuls co-execute
                    for w_t, rhs_t, st, sp in (
                        (w2h_t, hh_t, True, False),
                        (w2l_t, hh_t, False, False),
                        (w2h_t, hl_t, False, True),
                    ):
                        for par in range(2):
                            c0 = 64 * par
                            nc.tensor.matmul(
                                po[par][c0 : c0 + 64, :],
                                w_t[c0 : c0 + 64, :],
                                rhs_t[c0 : c0 + 64, :],
                                start=st, stop=sp, tile_position=(c0, c0),
                            )
                    # PSUM -> SBUF copies (b2 added on host)
                    grp, cg = divmod(pair, 2)
                    for par in range(2):
                        c0 = 64 * par
                        nc.scalar.activation(
                            ob_t[c0 : c0 + 64, grp, bass.ts(cg, NHALF)],
                            po[par][c0 : c0 + 64, :],
                            mybir.ActivationFunctionType.Copy,
                        )
                nc.sync.dma_start(out[blk], ob_t[:])

    nc.compile()
    return nc


def _get_program():
    if "prog" not in _PROGRAM_CACHE:
        _PROGRAM_CACHE["prog"] = _build_program()
    return _PROGRAM_CACHE["prog"]


def _pad_rows(a, n):
    if a.shape[0] == n:
        return a
    pad = np.zeros((n - a.shape[0],) + a.shape[1:], dtype=a.dtype)
    return np.concatenate([a, pad], axis=0)


def _split16(a):
    """fp32 array -> (half hi, half lo) with hi + lo ~= a."""
    hi = a.astype(_NP_HALF)
    lo = (a - hi.astype(np.float32)).astype(_NP_HALF)
    return hi, lo


def _host_pack(v_i, v_j, e_ij, W1, b1, W2, b2):
    """Build per-core input maps in the device layouts."""
    v_i = np.ascontiguousarray(v_i, dtype=np.float32)
    v_j = np.ascontiguousarray(v_j, dtype=np.float32)
    e_ij = np.ascontiguousarray(e_ij, dtype=np.float32)

    Wx = np.asarray(W1[:128], dtype=np.float32)
    We = np.asarray(W1[128:160], dtype=np.float32)
    Wxh, Wxl = _split16(Wx)
    Weh, Wel = _split16(We)
    W2h, W2l = _split16(np.asarray(W2, dtype=np.float32))

    es_w = np.concatenate([Weh, Wel, Weh], axis=0)  # [96, 64] halfword

    weights = {
        "w1a_h": np.ascontiguousarray(Wxh),
        "w1a_l": np.ascontiguousarray(Wxl),
        "w_es": np.ascontiguousarray(es_w),
        "w2h_r": np.ascontiguousarray(np.tile(W2h, (2, 1))),
        "w2l_r": np.ascontiguousarray(np.tile(W2l, (2, 1))),
        "b1r": np.ascontiguousarray(np.tile(b1, 2)[:, None], dtype=np.float32),
    }

    in_maps = []
    for c in range(N_CORES):
        sl = slice(c * EPC, (c + 1) * EPC)
        vi = _pad_rows(v_i[sl], EPAD)    # [EPAD, 64]
        vj = _pad_rows(v_j[sl], EPAD)
        ec = _pad_rows(e_ij[sl], EPAD)   # [EPAD, 32]

        # xta[b, p, h, n] = (Ah|Al)[p, b*4096 + n],  A = [v_i^T; v_j^T]
        A = np.concatenate([vi.T, vj.T], axis=0)          # [128, EPAD] f32
        Ah, Al = _split16(A)
        st = np.stack([Ah, Al], axis=1)                   # [128, 2, EPAD]
        xta = np.ascontiguousarray(
            st.reshape(128, 2, N_BLK, BLK_EDGES).transpose(2, 0, 1, 3)
        )  # [N_BLK, 128, 2, 4096] half

        # e-stack [eh; eh; el] along K at rows 0:96 for every q
        eh, el = _split16(ec)                             # [EPAD, 32] each
        EST = np.concatenate([eh, eh, el], axis=1).T      # [96, EPAD] f16
        Tr = EST.reshape(96, N_BLK, Q_PER_BLK, NHALF)     # [r, b, q, n]
        xtb = np.ascontiguousarray(Tr.transpose(1, 0, 2, 3))

        in_maps.append({"xta": xta, "xtb": xtb, **weights})
    return in_maps


def _host_unpack(results, b2):
    """results: list of per-core dicts with 'out' [N_BLK, 128, 2, 1024]."""
    b2 = np.asarray(b2, dtype=np.float32)
    outs = []
    for c in range(N_CORES):
        o = np.asarray(results[c]["out"])
        # out[b, 64*par + p, grp, 512*cg + n]
        #   = OUT[b*4096 + grp*2048 + cg*1024 + par*512 + n, p]
        r = o.reshape(N_BLK, 2, 64, 2, 2, NHALF)   # [b, par, p, grp, cg, n]
        r = r.transpose(0, 3, 4, 1, 5, 2)           # [b, grp, cg, par, n, p]
        outs.append(np.ascontiguousarray(r).reshape(EPAD, OUT_C)[:EPC] + b2)
    return np.concatenate(outs, axis=0)


def kernel(v_i, v_j, e_ij, W1, b1, W2, b2):
    global LAST_RESULT
    nc = _get_program()
    in_maps = _host_pack(v_i, v_j, e_ij, W1, b1, W2, b2)
    res = run_bass_kernel_spmd(
        nc, in_maps, core_ids=list(range(N_CORES)), trace=_TRACE
    )
    LAST_RESULT = res
    return _host_unpack(res.results, b2)
